# revision 1
# baseline (speedup 1.0000x reference)
"""DLRM (nn_DLRM_RPC) Trainium2 Bass kernel.

Strategy: pure data-parallel over batch across 8 NeuronCores; embedding
tables replicated in each core's HBM (966 MB bf16 total for all cores'
shares of device HBM - fits easily), so no collectives are needed.

Per core (2048 samples, 4 sample-tiles of 512):
  - one multi-index indirect DMA per 128-sample chunk gathers all 26
    embedding rows per sample (bf16, host-precast) into SBUF sample-major
  - PE transposes flip the gathered vectors feature-major into a grouped
    layout Eall[d, 123*g + 32*s + t]  (g = sample group of 4, t = slot:
    0 = bottom-MLP output x, 1..26 = embeddings)
  - bottom MLP runs feature-major and drops x straight into slot 0
  - per-group Gram matmuls B_g = blk^T @ blk give all 27x27 interaction
    dot products for 4 samples at once (diagonal s-blocks useful)
  - a partition-base-shifted copy pass rearranges Z into 7 K-stacked
    tiles Zstk_q[32u+j, b] = Z_b[4q+u, j]
  - top MLP consumes [x ; Zstk] with host-rearranged W0 (symmetric Z
    means only lower-triangle weights are placed), ReLU/Sigmoid fused
    into the PSUM->SBUF drains on the scalar engine.

All matmuls bf16 with fp32 PSUM accumulation.
"""

import os
import sys

import numpy as np

for _p in ("/opt/trn_rl_repo",):
    if _p not in sys.path and os.path.isdir(_p):
        sys.path.insert(0, _p)

import ml_dtypes

import concourse.bass as bass
import concourse.bacc as bacc
import concourse.mybir as mybir
import concourse.tile as tile
from concourse import bass_utils
from concourse.bass_interp import get_hw_module
from concourse.masks import make_identity

BF16 = ml_dtypes.bfloat16
F32 = np.float32

N_CORES = 8
B = 16384
SPC = B // N_CORES        # samples per core: 2048
NT = 27                   # slots: x + 26 tables
NE = 26
VOCAB = 50000
D = 128
BW = 123                  # group block width: 32*3 + 27
TS = 512                  # samples per tile
NTILES = SPC // TS        # 4
G = TS // 4               # groups per tile: 128
CH = TS // 128            # 128-sample chunks per tile: 4

LI, LJ = np.tril_indices(NT, -1)

_dt_bf16 = mybir.dt.bfloat16
_dt_f32 = mybir.dt.float32
_dt_i32 = mybir.dt.int32

_CACHE = {}


def _emit(tc, t):
    from contextlib import ExitStack

    nc = tc.nc
    Relu = mybir.ActivationFunctionType.Relu
    Sigmoid = mybir.ActivationFunctionType.Sigmoid

    with ExitStack() as ctx:
        sb = ctx.enter_context(tc.tile_pool(name="sb", bufs=1))
        db = ctx.enter_context(tc.tile_pool(name="db", bufs=2))
        mmps = ctx.enter_context(tc.tile_pool(name="mmps", bufs=2, space="PSUM"))
        grps = ctx.enter_context(tc.tile_pool(name="grps", bufs=2, space="PSUM"))
        trps = ctx.enter_context(tc.tile_pool(name="trps", bufs=2, space="PSUM"))
        w3ps = ctx.enter_context(tc.tile_pool(name="w3ps", bufs=1, space="PSUM"))

        ident = sb.tile([128, 128], _dt_bf16)
        make_identity(nc, ident[:])

        # --- load weights/inputs that stay resident ---
        def load(name, shape, dtype=_dt_bf16):
            tl = sb.tile(shape, dtype, name=name)
            nc.sync.dma_start(tl[:], t[name][:])
            return tl

        dxt = load("dxt", [16, SPC])
        bw0 = load("bw0", [16, 512])
        bb0 = load("bb0", [128, 4], _dt_f32)
        bw1 = load("bw1", [128, 4 * 256])
        bb1 = load("bb1", [128, 2], _dt_f32)
        bw2 = load("bw2", [128, 2 * 128])
        bb2 = load("bb2", [128, 1], _dt_f32)
        w0x = load("w0x", [128, 1024])
        wz = load("wz", [128, 7 * 1024])
        tb0 = load("tb0", [128, 8], _dt_f32)
        w1 = load("w1", [128, 8 * 1024])
        tb1 = load("tb1", [128, 8], _dt_f32)
        w2 = load("w2", [128, 8 * 512])
        tb2 = load("tb2", [128, 4], _dt_f32)
        w3 = load("w3", [128, 4])
        tb3 = load("tb3", [1, 1], _dt_f32)

        eall = sb.tile([128, BW * G], _dt_bf16)
        zsb = sb.tile([128, BW * G], _dt_bf16)
        zstk = [sb.tile([128, TS], _dt_bf16, name=f"zstk{q}") for q in range(7)]
        for q in range(7):
            nc.vector.memset(zstk[q][:], 0.0)

        eb = eall[:]
        pstep = eb.ap[0]
        # zero the 5 pad columns after each of the first 3 s-blocks
        pad_ap = bass.AP(eb.tensor, eb.offset + 27,
                         [pstep, [BW, G], [32, 3], [1, 5]])
        nc.vector.memset(pad_ap, 0.0)

        zb = zsb[:]
        zsb3 = zb.rearrange("p (g c) -> p g c", c=BW)

        for n in range(NTILES):
            # ---- Phase A: gather + transpose into Eall ----
            for c in range(CH):
                C = CH * n + c
                idxt = db.tile([128, NE], _dt_i32, name="idxt")
                nc.sync.dma_start(idxt[:], t["idx"][128 * C:128 * (C + 1), :])
                esm = db.tile([128, NE * D], _dt_bf16, name="esm")
                nc.gpsimd.indirect_dma_start(
                    out=esm[:], out_offset=None,
                    in_=t["tbl"][:],
                    in_offset=bass.IndirectOffsetOnAxis(ap=idxt[:], axis=0),
                )
                for t8 in range(4):
                    nt8 = 8 if t8 < 3 else 2
                    trp = trps.tile([128, 128 * nt8], _dt_bf16,
                                    name="trp", tag="trp")
                    for k in range(nt8):
                        ti = 8 * t8 + k
                        nc.tensor.transpose(
                            trp[:, 128 * k:128 * (k + 1)],
                            esm[:, 128 * ti:128 * (ti + 1)], ident[:])
                    dst = bass.AP(
                        eb.tensor, eb.offset + BW * 32 * c + 8 * t8 + 1,
                        [pstep, [1, nt8], [BW, 32], [32, 4]])
                    nc.vector.tensor_copy(dst, trp[:])

            # ---- Phase B: bottom MLP -> x into Eall slot 0 ----
            h0 = db.tile([128, 4 * 512], _dt_bf16, name="h0")
            for m in range(4):
                ps = mmps.tile([128, 512], _dt_f32, name="mm", tag="mm")
                nc.tensor.matmul(ps[:], bw0[:, 128 * m:128 * (m + 1)],
                                 dxt[:, TS * n:TS * (n + 1)],
                                 start=True, stop=True)
                nc.scalar.activation(h0[:, 512 * m:512 * (m + 1)], ps[:],
                                     Relu, bias=bb0[:, m:m + 1])
            h1b = db.tile([128, 2 * 512], _dt_bf16, name="h1b")
            for m in range(2):
                ps = mmps.tile([128, 512], _dt_f32, name="mm", tag="mm")
                for k in range(4):
                    nc.tensor.matmul(
                        ps[:], bw1[:, 256 * k + 128 * m:256 * k + 128 * (m + 1)],
                        h0[:, 512 * k:512 * (k + 1)],
                        start=(k == 0), stop=(k == 3))
                nc.scalar.activation(h1b[:, 512 * m:512 * (m + 1)], ps[:],
                                     Relu, bias=bb1[:, m:m + 1])
            ps = mmps.tile([128, 512], _dt_f32, name="mm", tag="mm")
            for k in range(2):
                nc.tensor.matmul(ps[:], bw2[:, 128 * k:128 * (k + 1)],
                                 h1b[:, 512 * k:512 * (k + 1)],
                                 start=(k == 0), stop=(k == 1))
            xdst = bass.AP(eb.tensor, eb.offset, [pstep, [BW, G], [32, 4]])
            nc.scalar.activation(xdst, ps[:], Relu, bias=bb2[:, 0:1])

            # ---- Phase C: Gram matmuls ----
            for r in range(G // 4):
                bank = grps.tile([128, 4 * BW], _dt_f32, name="grb", tag="gr")
                for k in range(4):
                    g = 4 * r + k
                    blk = eall[:, BW * g:BW * (g + 1)]
                    nc.tensor.matmul(bank[0:BW, BW * k:BW * (k + 1)],
                                     blk, blk, start=True, stop=True)
                nc.vector.tensor_copy(zsb[0:BW, 4 * BW * r:4 * BW * (r + 1)],
                                      bank[0:BW, :])

            # ---- Phase D: scramble Z into K-stacked tiles ----
            for i in range(NT):
                q, u = i // 4, i % 4
                for s in range(4):
                    src = zsb3[32 * s:32 * s + 27, :, 32 * s + i]
                    dst = zstk[q][:].rearrange("p (g s) -> p g s", s=4)[
                        32 * u:32 * u + 27, :, s]
                    nc.vector.tensor_copy(dst, src)

            # ---- Phase E: top MLP ----
            xap = bass.AP(eb.tensor, eb.offset, [pstep, [BW, G], [32, 4]])
            h1t = db.tile([128, 8 * 512], _dt_bf16, name="h1t")
            for m in range(8):
                ps = mmps.tile([128, 512], _dt_f32, name="mm", tag="mm")
                nc.tensor.matmul(ps[:], w0x[:, 128 * m:128 * (m + 1)], xap,
                                 start=True, stop=False)
                for q in range(7):
                    nc.tensor.matmul(
                        ps[:], wz[:, 1024 * q + 128 * m:1024 * q + 128 * (m + 1)],
                        zstk[q][:], start=False, stop=(q == 6))
                nc.scalar.activation(h1t[:, 512 * m:512 * (m + 1)], ps[:],
                                     Relu, bias=tb0[:, m:m + 1])
            h2t = db.tile([128, 8 * 512], _dt_bf16, name="h2t")
            for m in range(8):
                ps = mmps.tile([128, 512], _dt_f32, name="mm", tag="mm")
                for k in range(8):
                    nc.tensor.matmul(
                        ps[:], w1[:, 1024 * k + 128 * m:1024 * k + 128 * (m + 1)],
                        h1t[:, 512 * k:512 * (k + 1)],
                        start=(k == 0), stop=(k == 7))
                nc.scalar.activation(h2t[:, 512 * m:512 * (m + 1)], ps[:],
                                     Relu, bias=tb1[:, m:m + 1])
            h3t = db.tile([128, 4 * 512], _dt_bf16, name="h3t")
            for m in range(4):
                ps = mmps.tile([128, 512], _dt_f32, name="mm", tag="mm")
                for k in range(8):
                    nc.tensor.matmul(
                        ps[:], w2[:, 512 * k + 128 * m:512 * k + 128 * (m + 1)],
                        h2t[:, 512 * k:512 * (k + 1)],
                        start=(k == 0), stop=(k == 7))
                nc.scalar.activation(h3t[:, 512 * m:512 * (m + 1)], ps[:],
                                     Relu, bias=tb2[:, m:m + 1])
            ps3 = w3ps.tile([1, 512], _dt_f32, name="w3p", tag="w3")
            for k in range(4):
                nc.tensor.matmul(ps3[:], w3[:, k:k + 1],
                                 h3t[:, 512 * k:512 * (k + 1)],
                                 start=(k == 0), stop=(k == 3))
            outsb = db.tile([1, 512], _dt_f32, name="outsb")
            nc.scalar.activation(outsb[:], ps3[:], Sigmoid, bias=tb3[0:1, 0:1])
            nc.sync.dma_start(t["out"][n:n + 1, :], outsb[:])


def _build():
    if "nc" in _CACHE:
        return _CACHE["nc"]
    nc = bacc.Bacc("TRN2", target_bir_lowering=False, debug=False,
                   num_devices=N_CORES)
    t = {}

    def dram(name, shape, dt, kind="ExternalInput"):
        t[name] = nc.dram_tensor(name, shape, dt, kind=kind).ap()

    dram("tbl", [NE * VOCAB, D], _dt_bf16)
    dram("idx", [SPC, NE], _dt_i32)
    dram("dxt", [16, SPC], _dt_bf16)
    dram("bw0", [16, 512], _dt_bf16)
    dram("bb0", [128, 4], _dt_f32)
    dram("bw1", [128, 4 * 256], _dt_bf16)
    dram("bb1", [128, 2], _dt_f32)
    dram("bw2", [128, 2 * 128], _dt_bf16)
    dram("bb2", [128, 1], _dt_f32)
    dram("w0x", [128, 1024], _dt_bf16)
    dram("wz", [128, 7 * 1024], _dt_bf16)
    dram("tb0", [128, 8], _dt_f32)
    dram("w1", [128, 8 * 1024], _dt_bf16)
    dram("tb1", [128, 8], _dt_f32)
    dram("w2", [128, 8 * 512], _dt_bf16)
    dram("tb2", [128, 4], _dt_f32)
    dram("w3", [128, 4], _dt_bf16)
    dram("tb3", [1, 1], _dt_f32)
    dram("out", [NTILES, TS], _dt_f32, kind="ExternalOutput")

    with tile.TileContext(nc) as tc:
        _emit(tc, t)
    nc.compile()

    _CACHE["nc"] = nc
    return nc


def _ktile(w, kt, m):
    """[K, M] -> [128, (K//128) * M] with column kt*M + mm = w[128*kt + p, mm]."""
    K, Mo = w.shape
    return np.ascontiguousarray(
        w.reshape(K // 128, 128, Mo).transpose(1, 0, 2).reshape(128, -1))


def _shared_inputs(inputs):
    emb = np.asarray(inputs["emb_tables"])
    tbl = np.ascontiguousarray(
        emb.astype(BF16).reshape(NE * VOCAB, D))

    sh = {"tbl": tbl}
    sh["bw0"] = np.zeros((16, 512), BF16)
    sh["bw0"][:13] = np.asarray(inputs["bot_W0"]).astype(BF16)
    sh["bb0"] = np.asarray(inputs["bot_b0"]).astype(F32).reshape(4, 128).T.copy()
    sh["bw1"] = _ktile(np.asarray(inputs["bot_W1"]).astype(BF16), 4, 256)
    sh["bb1"] = np.asarray(inputs["bot_b1"]).astype(F32).reshape(2, 128).T.copy()
    sh["bw2"] = _ktile(np.asarray(inputs["bot_W2"]).astype(BF16), 2, 128)
    sh["bb2"] = np.asarray(inputs["bot_b2"]).astype(F32).reshape(1, 128).T.copy()

    w0 = np.asarray(inputs["top_W0"]).astype(F32)
    sh["w0x"] = w0[:128].astype(BF16)
    wgrid = np.zeros((NT, NT, 1024), F32)
    wgrid[LI, LJ] = w0[128:479]
    wz4 = np.zeros((7, 128, 1024), F32)
    for i in range(NT):
        q, u = i // 4, i % 4
        wz4[q, 32 * u:32 * u + NT] = wgrid[i]
    sh["wz"] = np.ascontiguousarray(
        wz4.transpose(1, 0, 2).reshape(128, 7 * 1024)).astype(BF16)
    sh["tb0"] = np.asarray(inputs["top_b0"]).astype(F32).reshape(8, 128).T.copy()
    sh["w1"] = _ktile(np.asarray(inputs["top_W1"]).astype(BF16), 8, 1024)
    sh["tb1"] = np.asarray(inputs["top_b1"]).astype(F32).reshape(8, 128).T.copy()
    sh["w2"] = _ktile(np.asarray(inputs["top_W2"]).astype(BF16), 8, 512)
    sh["tb2"] = np.asarray(inputs["top_b2"]).astype(F32).reshape(4, 128).T.copy()
    sh["w3"] = _ktile(np.asarray(inputs["top_W3"]).astype(BF16), 4, 1)
    sh["tb3"] = np.asarray(inputs["top_b3"]).astype(F32).reshape(1, 1)
    return sh


def _in_maps(inputs):
    sh = _shared_inputs(inputs)
    idx = np.asarray(inputs["indices"]).astype(np.int64)      # [26, B]
    gidx = (idx + (np.arange(NE) * VOCAB)[:, None]).astype(np.int32)
    dx = np.asarray(inputs["dense_x"]).astype(F32)            # [B, 13]
    maps = []
    for core in range(N_CORES):
        sl = slice(SPC * core, SPC * (core + 1))
        m = dict(sh)
        m["idx"] = np.ascontiguousarray(gidx[:, sl].T)        # [2048, 26]
        dxt = np.zeros((16, SPC), BF16)
        dxt[:13] = dx[sl].T.astype(BF16)
        m["dxt"] = dxt
        maps.append(m)
    return maps


def _run(inputs, trace=False):
    nc = _build()
    maps = _in_maps(inputs)
    old_m = nc.m
    nc.m = _CACHE.setdefault("hwm", get_hw_module(nc.m))
    try:
        res = bass_utils.run_bass_kernel_spmd(
            nc, maps, core_ids=list(range(N_CORES)), trace=trace)
    finally:
        nc.m = old_m
    out = np.concatenate([r["out"].reshape(-1) for r in res.results])
    return out.astype(F32).reshape(B, 1), res


def kernel(**inputs):
    out, _ = _run(inputs, trace=False)
    return out



# revision 6
# speedup vs baseline: 1.3831x; 1.3831x over previous
"""DLRM (nn_DLRM_RPC) Trainium2 Bass kernel — fp8 + contiguous-drain layout.

Strategy: pure data-parallel over batch across 8 NeuronCores; embedding
tables replicated (fp8, host-precast with x16 scale) so no collectives.

Per core (2048 samples, 4 tiles of 512, each tile = 4 chunks of 128):
  - indirect DMA gathers 26 fp8 embedding rows per sample, sample-major
  - PE transposes flip them feature-major; drains are fully CONTIGUOUS
    into eall[d, 4096*w + 128*slot + m]  (chunk w, slot 0 = x, 1..26 =
    tables, slots 27..31 zero padding)
  - bottom MLP (bf16) drops x*64 (fp8) into slot 0
  - Gram matmuls read eall via strided APs (group m = 4 samples strided
    across the 4 chunks, columns ordered 32w+i) -> Z for 4 samples/mm
  - Z lower triangle is packed into 3 K-chunks of 128 rows (ztop),
    alongside the x chunk -> top-MLP layer 0 has K=512 (not 1024)
  - top MLP in fp8 with DoubleRow (2 K-tiles per matmul), activations
    rescaled into fp8 range (h1*64, h2*256), last layer bf16 + sigmoid
  - PSUM->SBUF copies are rotated across vector/gpsimd/scalar engines;
    phases of adjacent tiles are interleaved to keep the PE warm (HAM)
"""

import os
import sys
from itertools import cycle

import numpy as np

for _p in ("/opt/trn_rl_repo",):
    if _p not in sys.path and os.path.isdir(_p):
        sys.path.insert(0, _p)

import ml_dtypes

import concourse.bass as bass
import concourse.bacc as bacc
import concourse.mybir as mybir
import concourse.tile as tile
from concourse import bass_utils
from concourse.bass_interp import get_hw_module
from concourse.masks import make_identity

BF16 = ml_dtypes.bfloat16
F8 = ml_dtypes.float8_e4m3
F32 = np.float32

N_CORES = 8
B = 16384
SPC = B // N_CORES        # samples per core: 2048
NT = 27                   # slots: x + 26 tables
NE = 26
VOCAB = 50000
D = 128
TS = 512                  # samples per tile
NTILES = SPC // TS        # 4
CPT = 4                   # chunks (of 128 samples) per tile
CHUNK = 32 * 128          # eall columns per chunk (32 slots, 5 padding)

# fp8 scale factors
S_EMB = 16.0              # tables stored as 16*emb
S_X = 64.0                # x stored as 64*x
S_H1 = 64.0
S_H2 = 256.0

LI, LJ = np.tril_indices(NT, -1)


# Z pair-run placement: run i -> chunk i//4, rows 32*(i%4)+j (j < i).
# 32-aligned row starts keep partition-base shifts at multiples of 32,
# which gpsimd's lane wiring requires for SBUF->SBUF copies.
PAIRQR = {i: (i // 4, 32 * (i % 4)) for i in range(1, NT)}
NZCH = 7                  # Z K-chunks for top layer 0

_dt_f8 = mybir.dt.float8e4
_dt_bf16 = mybir.dt.bfloat16
_dt_f32 = mybir.dt.float32
_dt_i32 = mybir.dt.int32

_CACHE = {}


def _emit(tc, t):
    from contextlib import ExitStack

    nc = tc.nc
    Relu = mybir.ActivationFunctionType.Relu
    Sigmoid = mybir.ActivationFunctionType.Sigmoid
    DR = mybir.MatmulPerfMode.DoubleRow

    with ExitStack() as ctx:
        sb = ctx.enter_context(tc.tile_pool(name="sb", bufs=1))
        db = ctx.enter_context(tc.tile_pool(name="db", bufs=2))
        esmp = ctx.enter_context(tc.tile_pool(name="esmp", bufs=8))
        mmps = ctx.enter_context(tc.tile_pool(name="mmps", bufs=2, space="PSUM"))
        grps = ctx.enter_context(tc.tile_pool(name="grps", bufs=2, space="PSUM"))
        trps = ctx.enter_context(tc.tile_pool(name="trps", bufs=2, space="PSUM"))
        w3ps = ctx.enter_context(tc.tile_pool(name="w3ps", bufs=1, space="PSUM"))

        ident = sb.tile([128, 128], _dt_f8)
        make_identity(nc, ident[:])

        def load(name, shape, dtype):
            tl = sb.tile(shape, dtype, name=name)
            nc.sync.dma_start(tl[:], t[name][:])
            return tl

        dxt = load("dxt", [16, SPC], _dt_bf16)
        bw0 = load("bw0", [16, 512], _dt_bf16)
        bb0 = load("bb0", [128, 4], _dt_f32)
        bw1 = load("bw1", [128, 4 * 256], _dt_bf16)
        bb1 = load("bb1", [128, 2], _dt_f32)
        bw2 = load("bw2", [128, 2 * 128], _dt_bf16)
        bb2 = load("bb2", [128, 1], _dt_f32)     # 64 * b2
        w0 = load("w0", [128, 8 * 1024], _dt_f8)
        tb0 = load("tb0", [128, 8], _dt_f32)     # 64 * top_b0
        w1 = load("w1", [128, 8 * 1024], _dt_f8)
        tb1 = load("tb1", [128, 8], _dt_f32)     # 256 * top_b1
        w2 = load("w2", [128, 8 * 512], _dt_f8)
        tb2 = load("tb2", [128, 4], _dt_f32)
        w3 = load("w3", [128, 4], _dt_bf16)
        tb3 = load("tb3", [1, 1], _dt_f32)

        w0r = w0[:].rearrange("p (k f) -> p k f", f=1024)
        w1r = w1[:].rearrange("p (k f) -> p k f", f=1024)
        w2r = w2[:].rearrange("p (k f) -> p k f", f=512)

        eall = [sb.tile([128, CPT * CHUNK], _dt_f8, name=f"eall{b_}")
                for b_ in range(2)]
        zsb = [sb.tile([128, 128 * 128], _dt_f8, name=f"zsb{b_}")
               for b_ in range(2)]
        ztop = [sb.tile([128, 8 * 512], _dt_f8, name=f"ztop{b_}")
                for b_ in range(2)]
        for b_ in range(2):
            e = eall[b_][:]
            pstep = e.ap[0]
            pad = bass.AP(e.tensor, e.offset + NT * 128,
                          [pstep, [CHUNK, CPT], [1, (32 - NT) * 128]])
            nc.vector.memset(pad, 0.0)
            nc.gpsimd.memset(ztop[b_][:], 0.0)

        # copy-engine rotations: gpsimd cannot touch PSUM, scalar is busy
        # with activations, so PSUM drains go to vector/scalar and pure
        # SBUF->SBUF moves go mostly to gpsimd.
        rot_ps = cycle([nc.vector, nc.vector, nc.scalar])
        rot_sb = cycle([nc.gpsimd, nc.gpsimd, nc.vector, nc.gpsimd,
                        nc.gpsimd, nc.scalar])

        def copy_ps(dst, src):
            e = next(rot_ps)
            if e is nc.scalar:
                e.copy(dst, src)
            else:
                e.tensor_copy(dst, src)

        def copy_sb(dst, src):
            e = next(rot_sb)
            if e is nc.scalar:
                e.copy(dst, src)
            else:
                e.tensor_copy(dst, src)

        esm_tiles = {}

        def emit_gather(n):
            for w in range(CPT):
                C = CPT * n + w
                idxt = esmp.tile([128, NE], _dt_i32, name="idxt", tag="idxt")
                nc.sync.dma_start(idxt[:], t["idx"][128 * C:128 * (C + 1), :])
                esm = esmp.tile([128, NE * D], _dt_f8, name="esm", tag="esm")
                nc.gpsimd.indirect_dma_start(
                    out=esm[:], out_offset=None,
                    in_=t["tbl"][:],
                    in_offset=bass.IndirectOffsetOnAxis(ap=idxt[:], axis=0),
                )
                esm_tiles[(n, w)] = esm

        def emit_trans(n):
            # transpose via regular fp8 matmul against identity (fp32 PSUM):
            # out[p, c] = esm[c, p]; avoids the fp8 transpose-mode step rule
            for w in range(CPT):
                esm = esm_tiles.pop((n, w))
                ti = 0
                while ti < NE:
                    ntr = min(4, NE - ti)
                    trp = trps.tile([128, 4 * 128], _dt_f32,
                                    name="trp", tag="trp")
                    for k in range(ntr):
                        nc.tensor.matmul(
                            trp[:, 128 * k:128 * (k + 1)],
                            esm[:, 128 * (ti + k):128 * (ti + k + 1)],
                            ident[:], start=True, stop=True)
                    base = CHUNK * w + 128 * (1 + ti)
                    copy_ps(eall[n % 2][:, base:base + ntr * 128],
                            trp[:, 0:ntr * 128])
                    ti += ntr

        def emit_bot(n):
            eb = eall[n % 2][:]
            pstep = eb.ap[0]
            h0 = db.tile([128, 4 * 512], _dt_bf16, name="h0", tag="h0")
            for m in range(4):
                ps = mmps.tile([128, 512], _dt_f32, name="mm", tag="mm")
                nc.tensor.matmul(ps[:], bw0[:, 128 * m:128 * (m + 1)],
                                 dxt[:, TS * n:TS * (n + 1)],
                                 start=True, stop=True)
                nc.scalar.activation(h0[:, 512 * m:512 * (m + 1)], ps[:],
                                     Relu, bias=bb0[:, m:m + 1])
            h1b = db.tile([128, 2 * 512], _dt_bf16, name="h1b", tag="h1b")
            for m in range(2):
                ps = mmps.tile([128, 512], _dt_f32, name="mm", tag="mm")
                for k in range(4):
                    nc.tensor.matmul(
                        ps[:], bw1[:, 256 * k + 128 * m:256 * k + 128 * (m + 1)],
                        h0[:, 512 * k:512 * (k + 1)],
                        start=(k == 0), stop=(k == 3))
                nc.scalar.activation(h1b[:, 512 * m:512 * (m + 1)], ps[:],
                                     Relu, bias=bb1[:, m:m + 1])
            ps = mmps.tile([128, 512], _dt_f32, name="mm", tag="mm")
            for k in range(2):
                nc.tensor.matmul(ps[:], bw2[:, 128 * k:128 * (k + 1)],
                                 h1b[:, 512 * k:512 * (k + 1)],
                                 start=(k == 0), stop=(k == 1))
            # x*64 -> fp8 into ztop chunk 0 (contiguous), then into eall slot 0
            zt = ztop[n % 2]
            nc.scalar.activation(zt[:, 0:512], ps[:], Relu,
                                 bias=bb2[:, 0:1], scale=S_X)
            xdst = bass.AP(eb.tensor, eb.offset, [pstep, [CHUNK, CPT], [1, 128]])
            copy_sb(xdst, zt[:, 0:512])

        def emit_gram(n):
            eb = eall[n % 2][:]
            pstep = eb.ap[0]
            for r in range(32):
                bank = grps.tile([128, 4 * 128], _dt_f32, name="grb", tag="gr")
                for j in range(4):
                    m = 4 * r + j
                    blk = bass.AP(eb.tensor, eb.offset + m,
                                  [pstep, [CHUNK, CPT], [128, 32]])
                    nc.tensor.matmul(bank[:, 128 * j:128 * (j + 1)],
                                     blk, blk, start=True, stop=True)
                copy_ps(zsb[n % 2][:, 512 * r:512 * (r + 1)], bank[:])

        def emit_scram(n):
            zsb3 = zsb[n % 2][:].rearrange("p (m c) -> p m c", c=128)
            zt = ztop[n % 2]
            for i in range(1, NT):
                q, r = PAIRQR[i]
                for w in range(CPT):
                    src = zsb3[32 * w:32 * w + i, :, 32 * w + i]
                    dst = zt[r:r + i,
                             512 * (1 + q) + 128 * w:512 * (1 + q) + 128 * (w + 1)]
                    copy_sb(dst, src)

        def emit_top(n):
            ztr = ztop[n % 2][:].rearrange("p (k f) -> p k f", f=512)
            h1t = db.tile([128, 8 * 512], _dt_f8, name="h1t", tag="h1t")
            h1r = h1t[:].rearrange("p (k f) -> p k f", f=512)
            for m in range(8):
                ps = mmps.tile([128, 512], _dt_f32, name="mm", tag="mm")
                for kp in range(4):
                    nc.tensor.matmul(
                        ps[:], w0r[:, 2 * kp:2 * kp + 2, 128 * m:128 * (m + 1)],
                        ztr[:, 2 * kp:2 * kp + 2, :],
                        start=(kp == 0), stop=(kp == 3), perf_mode=DR)
                nc.scalar.activation(h1t[:, 512 * m:512 * (m + 1)], ps[:],
                                     Relu, bias=tb0[:, m:m + 1], scale=S_H1)
            h2t = db.tile([128, 8 * 512], _dt_f8, name="h2t", tag="h2t")
            h2r = h2t[:].rearrange("p (k f) -> p k f", f=512)
            for m in range(8):
                ps = mmps.tile([128, 512], _dt_f32, name="mm", tag="mm")
                for kp in range(4):
                    nc.tensor.matmul(
                        ps[:], w1r[:, 2 * kp:2 * kp + 2, 128 * m:128 * (m + 1)],
                        h1r[:, 2 * kp:2 * kp + 2, :],
                        start=(kp == 0), stop=(kp == 3), perf_mode=DR)
                nc.scalar.activation(h2t[:, 512 * m:512 * (m + 1)], ps[:],
                                     Relu, bias=tb1[:, m:m + 1], scale=S_H2)
            h3t = db.tile([128, 4 * 512], _dt_bf16, name="h3t", tag="h3t")
            h3r = h3t[:].rearrange("p (k f) -> p k f", f=512)
            for m in range(4):
                ps = mmps.tile([128, 512], _dt_f32, name="mm", tag="mm")
                for kp in range(4):
                    nc.tensor.matmul(
                        ps[:], w2r[:, 2 * kp:2 * kp + 2, 128 * m:128 * (m + 1)],
                        h2r[:, 2 * kp:2 * kp + 2, :],
                        start=(kp == 0), stop=(kp == 3), perf_mode=DR)
                nc.scalar.activation(h3t[:, 512 * m:512 * (m + 1)], ps[:],
                                     Relu, bias=tb2[:, m:m + 1])
            ps3 = w3ps.tile([1, 512], _dt_f32, name="w3p", tag="w3")
            for k in range(4):
                nc.tensor.matmul(ps3[:], w3[:, k:k + 1],
                                 h3r[:, k, :],
                                 start=(k == 0), stop=(k == 3))
            outsb = db.tile([1, 512], _dt_f32, name="outsb", tag="outsb")
            nc.scalar.activation(outsb[:], ps3[:], Sigmoid, bias=tb3[0:1, 0:1])
            nc.sync.dma_start(t["out"][n:n + 1, :], outsb[:])

        emit_gather(0)
        emit_trans(0)
        emit_bot(0)
        for n in range(NTILES):
            if n + 1 < NTILES:
                emit_gather(n + 1)
            emit_gram(n)
            if n + 1 < NTILES:
                emit_trans(n + 1)
                emit_bot(n + 1)
            emit_scram(n)
            emit_top(n)


def _build():
    if "nc" in _CACHE:
        return _CACHE["nc"]
    nc = bacc.Bacc("TRN2", target_bir_lowering=False, debug=False,
                   num_devices=N_CORES)
    t = {}

    def dram(name, shape, dt, kind="ExternalInput"):
        t[name] = nc.dram_tensor(name, shape, dt, kind=kind).ap()

    dram("tbl", [NE * VOCAB, D], _dt_f8)
    dram("idx", [SPC, NE], _dt_i32)
    dram("dxt", [16, SPC], _dt_bf16)
    dram("bw0", [16, 512], _dt_bf16)
    dram("bb0", [128, 4], _dt_f32)
    dram("bw1", [128, 4 * 256], _dt_bf16)
    dram("bb1", [128, 2], _dt_f32)
    dram("bw2", [128, 2 * 128], _dt_bf16)
    dram("bb2", [128, 1], _dt_f32)
    dram("w0", [128, 8 * 1024], _dt_f8)
    dram("tb0", [128, 8], _dt_f32)
    dram("w1", [128, 8 * 1024], _dt_f8)
    dram("tb1", [128, 8], _dt_f32)
    dram("w2", [128, 8 * 512], _dt_f8)
    dram("tb2", [128, 4], _dt_f32)
    dram("w3", [128, 4], _dt_bf16)
    dram("tb3", [1, 1], _dt_f32)
    dram("out", [NTILES, TS], _dt_f32, kind="ExternalOutput")

    with tile.TileContext(nc) as tc:
        _emit(tc, t)
    nc.compile()

    _CACHE["nc"] = nc
    return nc


def _ktile(w, kt, m):
    """[K, M] -> [128, (K//128) * M] with column kt*M + mm = w[128*kt + p, mm]."""
    K, Mo = w.shape
    return np.ascontiguousarray(
        w.reshape(K // 128, 128, Mo).transpose(1, 0, 2).reshape(128, -1))


def _shared_inputs(inputs):
    emb = np.asarray(inputs["emb_tables"], F32)
    tbl = np.ascontiguousarray(
        (emb * S_EMB).astype(F8).reshape(NE * VOCAB, D))

    sh = {"tbl": tbl}
    sh["bw0"] = np.zeros((16, 512), BF16)
    sh["bw0"][:13] = np.asarray(inputs["bot_W0"]).astype(BF16)
    sh["bb0"] = np.asarray(inputs["bot_b0"]).astype(F32).reshape(4, 128).T.copy()
    sh["bw1"] = _ktile(np.asarray(inputs["bot_W1"]).astype(BF16), 4, 256)
    sh["bb1"] = np.asarray(inputs["bot_b1"]).astype(F32).reshape(2, 128).T.copy()
    sh["bw2"] = _ktile(np.asarray(inputs["bot_W2"]).astype(BF16), 2, 128)
    sh["bb2"] = (S_X * np.asarray(inputs["bot_b2"]).astype(F32)).reshape(1, 128).T.copy()

    # top layer 0: 4 K-chunks [x(scaled 64) ; 3 triangle-packed Z chunks]
    w0 = np.asarray(inputs["top_W0"]).astype(F32)
    w0kt = np.zeros((8, 128, 1024), F32)
    w0kt[0] = w0[:128] / (S_X * 1.0)
    for P in range(len(LI)):
        i, j = int(LI[P]), int(LJ[P])
        q, r = PAIRQR[i]
        # ztop values: <s_i*T_i, s_j*T_j> with s = S_X for slot 0, S_EMB else
        si = S_X if i == 0 else S_EMB
        sj = S_X if j == 0 else S_EMB
        w0kt[1 + q, r + j] = w0[128 + P] / (si * sj)
    sh["w0"] = np.ascontiguousarray(
        w0kt.transpose(1, 0, 2).reshape(128, 8 * 1024)).astype(F8)
    sh["tb0"] = (S_H1 * np.asarray(inputs["top_b0"]).astype(F32)).reshape(8, 128).T.copy()
    sh["w1"] = _ktile(np.asarray(inputs["top_W1"]).astype(F32) / S_H1, 8, 1024).astype(F8)
    sh["tb1"] = (S_H2 * np.asarray(inputs["top_b1"]).astype(F32)).reshape(8, 128).T.copy()
    sh["w2"] = _ktile(np.asarray(inputs["top_W2"]).astype(F32) / S_H2, 8, 512).astype(F8)
    sh["tb2"] = np.asarray(inputs["top_b2"]).astype(F32).reshape(4, 128).T.copy()
    sh["w3"] = _ktile(np.asarray(inputs["top_W3"]).astype(BF16), 4, 1)
    sh["tb3"] = np.asarray(inputs["top_b3"]).astype(F32).reshape(1, 1)
    return sh


def _in_maps(inputs):
    sh = _shared_inputs(inputs)
    idx = np.asarray(inputs["indices"]).astype(np.int64)      # [26, B]
    gidx = (idx + (np.arange(NE) * VOCAB)[:, None]).astype(np.int32)
    dx = np.asarray(inputs["dense_x"]).astype(F32)            # [B, 13]
    maps = []
    for core in range(N_CORES):
        sl = slice(SPC * core, SPC * (core + 1))
        m = dict(sh)
        m["idx"] = np.ascontiguousarray(gidx[:, sl].T)        # [2048, 26]
        dxt = np.zeros((16, SPC), BF16)
        dxt[:13] = dx[sl].T.astype(BF16)
        m["dxt"] = dxt
        maps.append(m)
    return maps


def _run(inputs, trace=False):
    nc = _build()
    maps = _in_maps(inputs)
    old_m = nc.m
    nc.m = _CACHE.setdefault("hwm", get_hw_module(nc.m))
    try:
        res = bass_utils.run_bass_kernel_spmd(
            nc, maps, core_ids=list(range(N_CORES)), trace=trace)
    finally:
        nc.m = old_m
    out = np.concatenate([r["out"].reshape(-1) for r in res.results])
    return out.astype(F32).reshape(B, 1), res


def kernel(**inputs):
    out, _ = _run(inputs, trace=False)
    return out


# revision 8
# speedup vs baseline: 1.6832x; 1.2170x over previous
"""DLRM (nn_DLRM_RPC) Trainium2 Bass kernel — fp8 + contiguous-drain layout.

Strategy: pure data-parallel over batch across 8 NeuronCores; embedding
tables replicated (fp8, host-precast with x16 scale) so no collectives.

Per core (2048 samples, 4 tiles of 512, each tile = 4 chunks of 128):
  - indirect DMA gathers 26 fp8 embedding rows per sample, sample-major
  - PE transposes flip them feature-major; drains are fully CONTIGUOUS
    into eall[d, 4096*w + 128*slot + m]  (chunk w, slot 0 = x, 1..26 =
    tables, slots 27..31 zero padding)
  - bottom MLP (bf16) drops x*64 (fp8) into slot 0
  - Gram matmuls read eall via strided APs (group m = 4 samples strided
    across the 4 chunks, columns ordered 32w+i) -> Z for 4 samples/mm
  - Z lower triangle is packed into 3 K-chunks of 128 rows (ztop),
    alongside the x chunk -> top-MLP layer 0 has K=512 (not 1024)
  - top MLP in fp8 with DoubleRow (2 K-tiles per matmul), activations
    rescaled into fp8 range (h1*64, h2*256), last layer bf16 + sigmoid
  - PSUM->SBUF copies are rotated across vector/gpsimd/scalar engines;
    phases of adjacent tiles are interleaved to keep the PE warm (HAM)
"""

import os
import sys
from itertools import cycle

import numpy as np

for _p in ("/opt/trn_rl_repo",):
    if _p not in sys.path and os.path.isdir(_p):
        sys.path.insert(0, _p)

import ml_dtypes

import concourse.bass as bass
import concourse.bacc as bacc
import concourse.mybir as mybir
import concourse.tile as tile
from concourse import bass_utils
from concourse.bass_interp import get_hw_module
from concourse.masks import make_identity

BF16 = ml_dtypes.bfloat16
F8 = ml_dtypes.float8_e4m3
F32 = np.float32

N_CORES = 8
B = 16384
SPC = B // N_CORES        # samples per core: 2048
NT = 27                   # slots: x + 26 tables
NE = 26
VOCAB = 50000
D = 128
TS = 512                  # samples per tile
NTILES = SPC // TS        # 4
CPT = 4                   # chunks (of 128 samples) per tile
CHUNK = 32 * 128          # eall columns per chunk (32 slots, 5 padding)

# fp8 scale factors
S_EMB = 16.0              # tables stored as 16*emb
S_X = 64.0                # x stored as 64*x
S_H1 = 64.0
S_H2 = 256.0

LI, LJ = np.tril_indices(NT, -1)


# Z pair-run placement: run i -> chunk i//4, rows 32*(i%4)+j (j < i).
# 32-aligned row starts keep partition-base shifts at multiples of 32,
# which gpsimd's lane wiring requires for SBUF->SBUF copies.
PAIRQR = {i: (i // 4, 32 * (i % 4)) for i in range(1, NT)}
NZCH = 7                  # Z K-chunks for top layer 0

_dt_f8 = mybir.dt.float8e4
_dt_bf16 = mybir.dt.bfloat16
_dt_f32 = mybir.dt.float32
_dt_i32 = mybir.dt.int32

_CACHE = {}


def _emit(tc, t):
    from contextlib import ExitStack

    nc = tc.nc
    Relu = mybir.ActivationFunctionType.Relu
    Sigmoid = mybir.ActivationFunctionType.Sigmoid
    DR = mybir.MatmulPerfMode.DoubleRow

    with ExitStack() as ctx:
        sb = ctx.enter_context(tc.tile_pool(name="sb", bufs=1))
        db = ctx.enter_context(tc.tile_pool(name="db", bufs=2))
        esmp = ctx.enter_context(tc.tile_pool(name="esmp", bufs=8))
        mmps = ctx.enter_context(tc.tile_pool(name="mmps", bufs=2, space="PSUM"))
        grps = ctx.enter_context(tc.tile_pool(name="grps", bufs=2, space="PSUM"))
        trps = ctx.enter_context(tc.tile_pool(name="trps", bufs=2, space="PSUM"))
        zkps = ctx.enter_context(tc.tile_pool(name="zkps", bufs=2, space="PSUM"))

        ident = sb.tile([128, 128], _dt_f8)
        make_identity(nc, ident[:])
        # identity shifted down 32 rows: idsh[k, c] = (k == c - 32); lets a
        # matmul write logical rows 96.. via out base 64 (base 96 is illegal)
        idsh = sb.tile([128, 192], _dt_f8)
        nc.gpsimd.memset(idsh[:], 0.0)
        make_identity(nc, idsh[:, 32:160], nomemset=True)

        def load(name, shape, dtype):
            tl = sb.tile(shape, dtype, name=name)
            nc.sync.dma_start(tl[:], t[name][:])
            return tl

        dxt = load("dxt", [16, SPC], _dt_bf16)
        bw0 = load("bw0", [16, 512], _dt_bf16)
        bb0 = load("bb0", [128, 4], _dt_f32)
        bw1 = load("bw1", [128, 4 * 256], _dt_bf16)
        bb1 = load("bb1", [128, 2], _dt_f32)
        bw2 = load("bw2", [128, 2 * 128], _dt_bf16)
        bb2 = load("bb2", [128, 1], _dt_f32)     # 64 * b2
        w0 = load("w0", [128, 8 * 1024], _dt_f8)
        tb0 = load("tb0", [128, 8], _dt_f32)     # 64 * top_b0
        w1 = load("w1", [128, 8 * 1024], _dt_f8)
        tb1 = load("tb1", [128, 8], _dt_f32)     # 256 * top_b1
        w2 = load("w2", [128, 8 * 512], _dt_f8)
        tb2 = load("tb2", [128, 4], _dt_f32)
        w3 = load("w3", [128, 4], _dt_bf16)
        tb3 = load("tb3", [1, 1], _dt_f32)

        w0r = w0[:].rearrange("p (k f) -> p k f", f=1024)
        w1r = w1[:].rearrange("p (k f) -> p k f", f=1024)
        w2r = w2[:].rearrange("p (k f) -> p k f", f=512)

        eall = [sb.tile([128, CPT * CHUNK], _dt_f8, name=f"eall{b_}")
                for b_ in range(2)]
        zsb = [sb.tile([128, 128 * 128], _dt_f8, name=f"zsb{b_}")
               for b_ in range(2)]
        ztop = [sb.tile([128, 8 * 512], _dt_f8, name=f"ztop{b_}")
                for b_ in range(2)]
        for b_ in range(2):
            e = eall[b_][:]
            pstep = e.ap[0]
            pad = bass.AP(e.tensor, e.offset + NT * 128,
                          [pstep, [CHUNK, CPT], [1, (32 - NT) * 128]])
            nc.vector.memset(pad, 0.0)
            nc.gpsimd.memset(ztop[b_][:], 0.0)

        # copy-engine rotations: gpsimd cannot touch PSUM, scalar is busy
        # with activations, so PSUM drains go to vector/scalar and pure
        # SBUF->SBUF moves go mostly to gpsimd.
        rot_ps = cycle([nc.vector, nc.vector, nc.vector, nc.scalar])
        rot_sb = cycle([nc.gpsimd])

        def copy_ps(dst, src):
            e = next(rot_ps)
            if e is nc.scalar:
                e.copy(dst, src)
            else:
                e.tensor_copy(dst, src)

        def copy_sb(dst, src):
            e = next(rot_sb)
            if e is nc.scalar:
                e.copy(dst, src)
            else:
                e.tensor_copy(dst, src)

        esm_tiles = {}

        def emit_gather(n):
            for w in range(CPT):
                C = CPT * n + w
                idxt = esmp.tile([128, NE], _dt_i32, name="idxt", tag="idxt")
                nc.sync.dma_start(idxt[:], t["idx"][128 * C:128 * (C + 1), :])
                esm = esmp.tile([128, NE * D], _dt_f8, name="esm", tag="esm")
                nc.gpsimd.indirect_dma_start(
                    out=esm[:], out_offset=None,
                    in_=t["tbl"][:],
                    in_offset=bass.IndirectOffsetOnAxis(ap=idxt[:], axis=0),
                )
                esm_tiles[(n, w)] = esm

        def emit_trans(n):
            # transpose via regular fp8 matmul against identity (fp32 PSUM):
            # out[p, c] = esm[c, p]; avoids the fp8 transpose-mode step rule
            for w in range(CPT):
                esm = esm_tiles.pop((n, w))
                ti = 0
                while ti < NE:
                    ntr = min(4, NE - ti)
                    trp = trps.tile([128, 4 * 128], _dt_f32,
                                    name="trp", tag="trp")
                    for k in range(ntr):
                        nc.tensor.matmul(
                            trp[:, 128 * k:128 * (k + 1)],
                            esm[:, 128 * (ti + k):128 * (ti + k + 1)],
                            ident[:], start=True, stop=True)
                    base = CHUNK * w + 128 * (1 + ti)
                    copy_ps(eall[n % 2][:, base:base + ntr * 128],
                            trp[:, 0:ntr * 128])
                    ti += ntr

        def emit_bot(n):
            eb = eall[n % 2][:]
            pstep = eb.ap[0]
            h0 = db.tile([128, 4 * 512], _dt_bf16, name="h0", tag="h0")
            for m in range(4):
                ps = mmps.tile([128, 512], _dt_f32, name="mm", tag="mm")
                nc.tensor.matmul(ps[:], bw0[:, 128 * m:128 * (m + 1)],
                                 dxt[:, TS * n:TS * (n + 1)],
                                 start=True, stop=True)
                nc.scalar.activation(h0[:, 512 * m:512 * (m + 1)], ps[:],
                                     Relu, bias=bb0[:, m:m + 1])
            h1b = db.tile([128, 2 * 512], _dt_bf16, name="h1b", tag="h1b")
            for m in range(2):
                ps = mmps.tile([128, 512], _dt_f32, name="mm", tag="mm")
                for k in range(4):
                    nc.tensor.matmul(
                        ps[:], bw1[:, 256 * k + 128 * m:256 * k + 128 * (m + 1)],
                        h0[:, 512 * k:512 * (k + 1)],
                        start=(k == 0), stop=(k == 3))
                nc.scalar.activation(h1b[:, 512 * m:512 * (m + 1)], ps[:],
                                     Relu, bias=bb1[:, m:m + 1])
            ps = mmps.tile([128, 512], _dt_f32, name="mm", tag="mm")
            for k in range(2):
                nc.tensor.matmul(ps[:], bw2[:, 128 * k:128 * (k + 1)],
                                 h1b[:, 512 * k:512 * (k + 1)],
                                 start=(k == 0), stop=(k == 1))
            # x*64 -> fp8 into ztop chunk 0 (contiguous), then into eall slot 0
            zt = ztop[n % 2]
            nc.scalar.activation(zt[:, 0:512], ps[:], Relu,
                                 bias=bb2[:, 0:1], scale=S_X)
            xdst = bass.AP(eb.tensor, eb.offset, [pstep, [CHUNK, CPT], [1, 128]])
            copy_sb(xdst, zt[:, 0:512])

        def emit_gram(n):
            eb = eall[n % 2][:]
            pstep = eb.ap[0]
            for r in range(32):
                bank = grps.tile([128, 4 * 128], _dt_f32, name="grb", tag="gr")
                for j in range(4):
                    m = 4 * r + j
                    blk = bass.AP(eb.tensor, eb.offset + m,
                                  [pstep, [CHUNK, CPT], [128, 32]])
                    nc.tensor.matmul(bank[:, 128 * j:128 * (j + 1)],
                                     blk, blk, start=True, stop=True)
                copy_ps(zsb[n % 2][:, 512 * r:512 * (r + 1)], bank[:])

        def emit_scram(n):
            # PE selection matmuls: zk[32u+j, 128w+m] = zsb[32w+j, 128m+32w+i]
            # (= Z_(w,m)[i, j] by Gram symmetry), one PSUM bank per Z chunk
            zs = zsb[n % 2][:]
            pstep = zs.ap[0]
            zt = ztop[n % 2]
            for q in range(NZCH):
                zk = zkps.tile([128, 512], _dt_f32, name="zk", tag="zk")
                if n == 0 and q < 2:
                    nc.vector.memset(zk[:], 0.0)
                for u in (3, 0, 1, 2):       # u=3 first: its widened write
                    i = 4 * q + u             # zeroes rows 64-95, u=2 redoes
                    if not 1 <= i < NT:
                        continue
                    for w in range(CPT):
                        rhs = bass.AP(zs.tensor, zs.offset + 32 * w + i,
                                      [pstep, [128, 128]])
                        if u == 3:
                            nc.tensor.matmul(
                                zk[64:96 + i, 128 * w:128 * (w + 1)],
                                idsh[:, 32 * w:32 * w + 32 + i], rhs,
                                start=True, stop=True)
                        else:
                            nc.tensor.matmul(
                                zk[32 * u:32 * u + i, 128 * w:128 * (w + 1)],
                                ident[:, 32 * w:32 * w + i], rhs,
                                start=True, stop=True)
                copy_ps(zt[:, 512 * (1 + q):512 * (2 + q)], zk[:])

        def emit_top(n):
            ztr = ztop[n % 2][:].rearrange("p (k f) -> p k f", f=512)
            h1t = db.tile([128, 8 * 512], _dt_f8, name="h1t", tag="h1t")
            h1r = h1t[:].rearrange("p (k f) -> p k f", f=512)
            for m in range(8):
                ps = mmps.tile([128, 512], _dt_f32, name="mm", tag="mm")
                for kp in range(4):
                    nc.tensor.matmul(
                        ps[:], w0r[:, 2 * kp:2 * kp + 2, 128 * m:128 * (m + 1)],
                        ztr[:, 2 * kp:2 * kp + 2, :],
                        start=(kp == 0), stop=(kp == 3), perf_mode=DR)
                nc.scalar.activation(h1t[:, 512 * m:512 * (m + 1)], ps[:],
                                     Relu, bias=tb0[:, m:m + 1], scale=S_H1)
            h2t = db.tile([128, 8 * 512], _dt_f8, name="h2t", tag="h2t")
            h2r = h2t[:].rearrange("p (k f) -> p k f", f=512)
            for m in range(8):
                ps = mmps.tile([128, 512], _dt_f32, name="mm", tag="mm")
                for kp in range(4):
                    nc.tensor.matmul(
                        ps[:], w1r[:, 2 * kp:2 * kp + 2, 128 * m:128 * (m + 1)],
                        h1r[:, 2 * kp:2 * kp + 2, :],
                        start=(kp == 0), stop=(kp == 3), perf_mode=DR)
                nc.scalar.activation(h2t[:, 512 * m:512 * (m + 1)], ps[:],
                                     Relu, bias=tb1[:, m:m + 1], scale=S_H2)
            h3t = db.tile([128, 4 * 512], _dt_bf16, name="h3t", tag="h3t")
            h3r = h3t[:].rearrange("p (k f) -> p k f", f=512)
            for m in range(4):
                ps = mmps.tile([128, 512], _dt_f32, name="mm", tag="mm")
                for kp in range(4):
                    nc.tensor.matmul(
                        ps[:], w2r[:, 2 * kp:2 * kp + 2, 128 * m:128 * (m + 1)],
                        h2r[:, 2 * kp:2 * kp + 2, :],
                        start=(kp == 0), stop=(kp == 3), perf_mode=DR)
                nc.scalar.activation(h3t[:, 512 * m:512 * (m + 1)], ps[:],
                                     Relu, bias=tb2[:, m:m + 1])
            ps3 = mmps.tile([128, 512], _dt_f32, name="mm", tag="mm")
            for k in range(4):
                nc.tensor.matmul(ps3[0:1, :], w3[:, k:k + 1],
                                 h3r[:, k, :],
                                 start=(k == 0), stop=(k == 3))
            outsb = db.tile([1, 512], _dt_f32, name="outsb", tag="outsb")
            nc.scalar.activation(outsb[:], ps3[0:1, :], Sigmoid, bias=tb3[0:1, 0:1])
            nc.sync.dma_start(t["out"][n:n + 1, :], outsb[:])

        emit_gather(0)
        emit_trans(0)
        emit_bot(0)
        for n in range(NTILES):
            if n + 1 < NTILES:
                emit_gather(n + 1)
            emit_gram(n)
            if n + 1 < NTILES:
                emit_trans(n + 1)
                emit_bot(n + 1)
            emit_scram(n)
            emit_top(n)


def _build():
    if "nc" in _CACHE:
        return _CACHE["nc"]
    nc = bacc.Bacc("TRN2", target_bir_lowering=False, debug=False,
                   num_devices=N_CORES)
    t = {}

    def dram(name, shape, dt, kind="ExternalInput"):
        t[name] = nc.dram_tensor(name, shape, dt, kind=kind).ap()

    dram("tbl", [NE * VOCAB, D], _dt_f8)
    dram("idx", [SPC, NE], _dt_i32)
    dram("dxt", [16, SPC], _dt_bf16)
    dram("bw0", [16, 512], _dt_bf16)
    dram("bb0", [128, 4], _dt_f32)
    dram("bw1", [128, 4 * 256], _dt_bf16)
    dram("bb1", [128, 2], _dt_f32)
    dram("bw2", [128, 2 * 128], _dt_bf16)
    dram("bb2", [128, 1], _dt_f32)
    dram("w0", [128, 8 * 1024], _dt_f8)
    dram("tb0", [128, 8], _dt_f32)
    dram("w1", [128, 8 * 1024], _dt_f8)
    dram("tb1", [128, 8], _dt_f32)
    dram("w2", [128, 8 * 512], _dt_f8)
    dram("tb2", [128, 4], _dt_f32)
    dram("w3", [128, 4], _dt_bf16)
    dram("tb3", [1, 1], _dt_f32)
    dram("out", [NTILES, TS], _dt_f32, kind="ExternalOutput")

    with tile.TileContext(nc) as tc:
        _emit(tc, t)
    nc.compile()

    _CACHE["nc"] = nc
    return nc


def _ktile(w, kt, m):
    """[K, M] -> [128, (K//128) * M] with column kt*M + mm = w[128*kt + p, mm]."""
    K, Mo = w.shape
    return np.ascontiguousarray(
        w.reshape(K // 128, 128, Mo).transpose(1, 0, 2).reshape(128, -1))


def _shared_inputs(inputs):
    emb = np.asarray(inputs["emb_tables"], F32)
    tbl = np.ascontiguousarray(
        (emb * S_EMB).astype(F8).reshape(NE * VOCAB, D))

    sh = {"tbl": tbl}
    sh["bw0"] = np.zeros((16, 512), BF16)
    sh["bw0"][:13] = np.asarray(inputs["bot_W0"]).astype(BF16)
    sh["bb0"] = np.asarray(inputs["bot_b0"]).astype(F32).reshape(4, 128).T.copy()
    sh["bw1"] = _ktile(np.asarray(inputs["bot_W1"]).astype(BF16), 4, 256)
    sh["bb1"] = np.asarray(inputs["bot_b1"]).astype(F32).reshape(2, 128).T.copy()
    sh["bw2"] = _ktile(np.asarray(inputs["bot_W2"]).astype(BF16), 2, 128)
    sh["bb2"] = (S_X * np.asarray(inputs["bot_b2"]).astype(F32)).reshape(1, 128).T.copy()

    # top layer 0: 4 K-chunks [x(scaled 64) ; 3 triangle-packed Z chunks]
    w0 = np.asarray(inputs["top_W0"]).astype(F32)
    w0kt = np.zeros((8, 128, 1024), F32)
    w0kt[0] = w0[:128] / (S_X * 1.0)
    for P in range(len(LI)):
        i, j = int(LI[P]), int(LJ[P])
        q, r = PAIRQR[i]
        # ztop values: <s_i*T_i, s_j*T_j> with s = S_X for slot 0, S_EMB else
        si = S_X if i == 0 else S_EMB
        sj = S_X if j == 0 else S_EMB
        w0kt[1 + q, r + j] = w0[128 + P] / (si * sj)
    sh["w0"] = np.ascontiguousarray(
        w0kt.transpose(1, 0, 2).reshape(128, 8 * 1024)).astype(F8)
    sh["tb0"] = (S_H1 * np.asarray(inputs["top_b0"]).astype(F32)).reshape(8, 128).T.copy()
    sh["w1"] = _ktile(np.asarray(inputs["top_W1"]).astype(F32) / S_H1, 8, 1024).astype(F8)
    sh["tb1"] = (S_H2 * np.asarray(inputs["top_b1"]).astype(F32)).reshape(8, 128).T.copy()
    sh["w2"] = _ktile(np.asarray(inputs["top_W2"]).astype(F32) / S_H2, 8, 512).astype(F8)
    sh["tb2"] = np.asarray(inputs["top_b2"]).astype(F32).reshape(4, 128).T.copy()
    sh["w3"] = _ktile(np.asarray(inputs["top_W3"]).astype(BF16), 4, 1)
    sh["tb3"] = np.asarray(inputs["top_b3"]).astype(F32).reshape(1, 1)
    return sh


def _in_maps(inputs):
    sh = _shared_inputs(inputs)
    idx = np.asarray(inputs["indices"]).astype(np.int64)      # [26, B]
    gidx = (idx + (np.arange(NE) * VOCAB)[:, None]).astype(np.int32)
    dx = np.asarray(inputs["dense_x"]).astype(F32)            # [B, 13]
    maps = []
    for core in range(N_CORES):
        sl = slice(SPC * core, SPC * (core + 1))
        m = dict(sh)
        m["idx"] = np.ascontiguousarray(gidx[:, sl].T)        # [2048, 26]
        dxt = np.zeros((16, SPC), BF16)
        dxt[:13] = dx[sl].T.astype(BF16)
        m["dxt"] = dxt
        maps.append(m)
    return maps


def _run(inputs, trace=False):
    nc = _build()
    maps = _in_maps(inputs)
    old_m = nc.m
    nc.m = _CACHE.setdefault("hwm", get_hw_module(nc.m))
    try:
        res = bass_utils.run_bass_kernel_spmd(
            nc, maps, core_ids=list(range(N_CORES)), trace=trace)
    finally:
        nc.m = old_m
    out = np.concatenate([r["out"].reshape(-1) for r in res.results])
    return out.astype(F32).reshape(B, 1), res


def kernel(**inputs):
    out, _ = _run(inputs, trace=False)
    return out
